# revision 4
# baseline (speedup 1.0000x reference)
"""CamCenterLoss (segment-mean SmoothL1) on 8 Trainium2 NeuronCores.

Sharding strategy: shard by camera id (there are exactly 8 cameras and 8
cores), so every (label, cam) segment is fully local to one core and no
collective is needed. On the host we sort each core's rows by label and pack
whole label-runs into 128-row blocks; each block's labels then span < 128
consecutive values, so segment sums/means/targets are computed with small
per-block one-hot matmuls on the TensorEngine.

Per block b (128 rows, feature dim split in halves of 1024):
  sums_b   = W_sum_b.T @ feats_b                (PE, bf16 operands, f32 PSUM)
  means_b  = sums_b * recip_b                   (ACT or DVE, per-partition scale)
  d_b      = W_exp_b.T @ means_b - feats_b      (PE, accumulated in PSUM)
  a        = |d|          -> Sum_a  (ACT Abs + accum)
  m        = min(a, 1)    -> Sum_m  (DVE tensor_scalar + accum)
  0.5*m*m                 -> Sum_mm (DVE tensor_tensor_reduce)
SmoothL1 identity:  sl1(d) = 0.5*min(|d|,1)^2 + |d| - min(|d|,1)
  => total = Sum_a - Sum_m + Sum_mm ; loss = total / (N*D)
"""

import numpy as np
import ml_dtypes

NUM_CAMS = 8
N_CORES = 8
F_HALF = 1024
QCHUNK = 512


# ----------------------------------------------------------------------------
# Host-side preprocessing (index/int manipulation + row permutation only)
# ----------------------------------------------------------------------------

def _pack_core(rows, labs):
    """Greedy-pack whole label runs into blocks of <=128 rows whose label
    span is < 128. Returns list of (row_idx_array, local_slot_array, base)."""
    blocks = []
    n = len(rows)
    if n == 0:
        return blocks
    # run boundaries
    starts = np.flatnonzero(np.r_[True, labs[1:] != labs[:-1]])
    ends = np.r_[starts[1:], n]
    cur_s = None  # start row index of current block
    cur_used = 0
    cur_base = 0
    for s, e in zip(starts, ends):
        L = int(labs[s])
        rl = e - s
        if rl > 128:
            raise ValueError("label run longer than 128 rows not supported")
        if cur_s is None:
            cur_s, cur_used, cur_base = s, 0, L
        elif cur_used + rl > 128 or (L - cur_base) >= 128:
            blocks.append((rows[cur_s:cur_s + cur_used],
                           labs[cur_s:cur_s + cur_used] - cur_base, cur_base))
            cur_s, cur_used, cur_base = s, 0, L
        cur_used += rl
    if cur_s is not None and cur_used > 0:
        blocks.append((rows[cur_s:cur_s + cur_used],
                       labs[cur_s:cur_s + cur_used] - cur_base, cur_base))
    return blocks


def _preprocess(feats, labels, cam_ids):
    feats = np.ascontiguousarray(np.asarray(feats, dtype=np.float32))
    labels = np.asarray(labels).astype(np.int64)
    cams = np.asarray(cam_ids).astype(np.int64)
    N, D = feats.shape

    per_core_blocks = []
    for c in range(N_CORES):
        rows = np.flatnonzero(cams == c)
        order = np.argsort(labels[rows], kind="stable")
        rows = rows[order]
        labs = labels[rows]
        per_core_blocks.append(_pack_core(rows, labs))

    nblk = max(len(b) for b in per_core_blocks)
    nblk = max(nblk, 1)

    bf16 = ml_dtypes.bfloat16
    feats_s = np.zeros((N_CORES, nblk * 128, D), dtype=bf16)
    w_sum = np.zeros((N_CORES, nblk, 128, 128), dtype=bf16)
    w_exp = np.zeros((N_CORES, nblk, 128, 128), dtype=bf16)
    recip = np.ones((N_CORES, 128, nblk), dtype=np.float32)

    for c in range(N_CORES):
        for b, (ridx, slot, _base) in enumerate(per_core_blocks[c]):
            k = len(ridx)
            feats_s[c, 128 * b:128 * b + k] = feats[ridx]
            ar = np.arange(k)
            w_sum[c, b, ar, slot] = 1
            w_exp[c, b, slot, ar] = 1
            cnt = np.bincount(slot, minlength=128)
            recip[c, :, b] = np.float32(1.0) / np.maximum(cnt, 1).astype(np.float32)

    neg_ident = (-np.eye(128)).astype(bf16)
    return feats_s, w_sum, w_exp, recip, neg_ident, nblk, N, D


# ----------------------------------------------------------------------------
# Device program
# ----------------------------------------------------------------------------

def _build_program(nblk, D):
    import concourse.bacc as bacc
    import concourse.mybir as mybir
    import concourse.tile as tile

    dt = mybir.dt
    f32, bf16 = dt.float32, dt.bfloat16
    Alu = mybir.AluOpType
    Act = mybir.ActivationFunctionType
    n_half = D // F_HALF
    ncols = nblk * n_half

    nc = bacc.Bacc("TRN2", target_bir_lowering=False, debug=False,
                   num_devices=N_CORES)
    feats_d = nc.dram_tensor("feats_s", [nblk * 128, D], bf16,
                             kind="ExternalInput").ap()
    wsum_d = nc.dram_tensor("w_sum", [nblk, 128, 128], bf16,
                            kind="ExternalInput").ap()
    wexp_d = nc.dram_tensor("w_exp", [nblk, 128, 128], bf16,
                            kind="ExternalInput").ap()
    recip_d = nc.dram_tensor("recip", [128, nblk], f32,
                             kind="ExternalInput").ap()
    nident_d = nc.dram_tensor("neg_ident", [128, 128], bf16,
                              kind="ExternalInput").ap()
    out_d = nc.dram_tensor("partial", [1, 1], f32, kind="ExternalOutput").ap()

    with tile.TileContext(nc) as tc:
        with (
            tc.tile_pool(name="const", bufs=1) as const_pool,
            tc.tile_pool(name="feats", bufs=5) as feats_pool,
            tc.tile_pool(name="wts", bufs=4) as wts_pool,
            tc.tile_pool(name="means", bufs=3) as means_pool,
            tc.tile_pool(name="aa", bufs=3) as a_pool,
            tc.tile_pool(name="mm", bufs=3) as m_pool,
            tc.tile_pool(name="scr", bufs=2) as scr_pool,
            tc.tile_pool(name="psums", bufs=2, space="PSUM") as psum_s_pool,
            tc.tile_pool(name="psumd", bufs=2, space="PSUM") as psum_d_pool,
        ):
            negI = const_pool.tile([128, 128], bf16, tag="negI")
            nc.sync.dma_start(negI[:], nident_d[:])
            recip_sb = const_pool.tile([128, nblk], f32, tag="recip")
            nc.sync.dma_start(recip_sb[:], recip_d[:])
            stats_a = const_pool.tile([128, ncols], f32, tag="stats_a")
            stats_m = const_pool.tile([128, ncols], f32, tag="stats_m")
            stats_mm = const_pool.tile([128, ncols], f32, tag="stats_mm")
            ones = const_pool.tile([128, 1], f32, tag="ones")
            nc.gpsimd.memset(ones[:], 1.0)

            for b in range(nblk):
                fe = feats_pool.tile([128, D], bf16, tag="fe")
                nc.sync.dma_start(fe[:], feats_d[128 * b:128 * (b + 1), :])
                ws = wts_pool.tile([128, 128], bf16, tag="ws")
                nc.sync.dma_start(ws[:], wsum_d[b])
                we = wts_pool.tile([128, 128], bf16, tag="we")
                nc.sync.dma_start(we[:], wexp_d[b])

                for h in range(n_half):
                    col = b * n_half + h
                    sums = psum_s_pool.tile([128, F_HALF], f32, tag="sums")
                    for q in range(F_HALF // QCHUNK):
                        lo = h * F_HALF + q * QCHUNK
                        nc.tensor.matmul(
                            sums[:, q * QCHUNK:(q + 1) * QCHUNK],
                            ws[:], fe[:, lo:lo + QCHUNK],
                            start=True, stop=True)

                    means = means_pool.tile([128, F_HALF], bf16, tag="means")
                    rap = recip_sb[:, b:b + 1]
                    if col % 2 == 0:
                        nc.scalar.activation(means[:], sums[:], Act.Copy,
                                             bias=0.0, scale=rap)
                    else:
                        nc.vector.tensor_scalar(means[:], sums[:], rap, None,
                                                op0=Alu.mult)

                    dps = psum_d_pool.tile([128, F_HALF], f32, tag="d")
                    for q in range(F_HALF // QCHUNK):
                        lo = h * F_HALF + q * QCHUNK
                        sl = slice(q * QCHUNK, (q + 1) * QCHUNK)
                        nc.tensor.matmul(dps[:, sl], we[:],
                                         means[:, sl],
                                         start=True, stop=False)
                        nc.tensor.matmul(dps[:, sl], negI[:],
                                         fe[:, lo:lo + QCHUNK],
                                         start=False, stop=True)

                    a = a_pool.tile([128, F_HALF], bf16, tag="a")
                    nc.scalar.activation(a[:], dps[:], Act.Abs,
                                         accum_out=stats_a[:, col:col + 1])
                    m = m_pool.tile([128, F_HALF], bf16, tag="m")
                    nc.vector.tensor_scalar(m[:], a[:], 1.0, None,
                                            op0=Alu.min, op1=Alu.add,
                                            accum_out=stats_m[:, col:col + 1])
                    sc = scr_pool.tile([128, F_HALF], bf16, tag="sc")
                    nc.vector.scalar_tensor_tensor(
                        sc[:], m[:], 0.5, m[:],
                        op0=Alu.mult, op1=Alu.mult,
                        accum_out=stats_mm[:, col:col + 1])

            # final: comb = red_a - red_m + red_mm ; partial = sum_p comb[p]
            red_a = const_pool.tile([128, 1], f32, tag="red_a")
            nc.vector.tensor_reduce(red_a[:], stats_a[:],
                                    axis=mybir.AxisListType.X, op=Alu.add)
            red_m = const_pool.tile([128, 1], f32, tag="red_m")
            nc.vector.tensor_reduce(red_m[:], stats_m[:],
                                    axis=mybir.AxisListType.X, op=Alu.add)
            red_mm = const_pool.tile([128, 1], f32, tag="red_mm")
            nc.vector.tensor_reduce(red_mm[:], stats_mm[:],
                                    axis=mybir.AxisListType.X, op=Alu.add)
            comb1 = const_pool.tile([128, 1], f32, tag="comb1")
            nc.vector.scalar_tensor_tensor(comb1[:], red_a[:], 1.0, red_m[:],
                                           op0=Alu.mult, op1=Alu.subtract)
            comb = const_pool.tile([128, 1], f32, tag="comb")
            nc.vector.scalar_tensor_tensor(comb[:], comb1[:], 1.0, red_mm[:],
                                           op0=Alu.mult, op1=Alu.add)
            fin = psum_s_pool.tile([1, 1], f32, tag="sums")
            nc.tensor.matmul(fin[:], comb[:], ones[:], start=True, stop=True)
            outsb = const_pool.tile([1, 1], f32, tag="outsb")
            nc.scalar.copy(outsb[:], fin[:])
            nc.sync.dma_start(out_d[:], outsb[:])

    nc.compile()
    return nc


_PROGRAM_CACHE = {}


def _get_program(nblk, D):
    key = (nblk, D)
    if key not in _PROGRAM_CACHE:
        _PROGRAM_CACHE[key] = _build_program(nblk, D)
    return _PROGRAM_CACHE[key]


def make_in_maps(feats, labels, cam_ids):
    """Host shard + program build; returns (nc, in_maps, N, D)."""
    feats_s, w_sum, w_exp, recip, neg_ident, nblk, N, D = _preprocess(
        feats, labels, cam_ids)
    nc = _get_program(nblk, D)
    in_maps = [
        {
            "feats_s": feats_s[c],
            "w_sum": w_sum[c],
            "w_exp": w_exp[c],
            "recip": recip[c],
            "neg_ident": neg_ident,
        }
        for c in range(N_CORES)
    ]
    return nc, in_maps, N, D


def kernel(feats, labels, cam_ids):
    from concourse.bass_utils import run_bass_kernel_spmd

    nc, in_maps, N, D = make_in_maps(feats, labels, cam_ids)
    res = run_bass_kernel_spmd(nc, in_maps, core_ids=list(range(N_CORES)))
    total = np.sum(
        np.array([res.results[c]["partial"][0, 0] for c in range(N_CORES)],
                 dtype=np.float64))
    return np.float32(total / (float(N) * float(D)))


# revision 7
# speedup vs baseline: 1.2224x; 1.2224x over previous
"""CamCenterLoss (segment-mean SmoothL1) on 8 Trainium2 NeuronCores.

Sharding strategy: shard by camera id (there are exactly 8 cameras and 8
cores), so every (label, cam) segment is fully local to one core and no
collective is needed. On the host we sort each core's rows by label and pack
whole label-runs into 128-row blocks; each block's labels then span < 128
consecutive values, so segment sums/means/targets are computed with small
per-block one-hot matmuls on the TensorEngine.

Per block b (128 rows, feature dim split in halves of 1024):
  sums_b   = W_sum_b.T @ feats_b                (PE, bf16 operands, f32 PSUM)
  means_b  = sums_b * recip_b                   (ACT or DVE, per-partition scale)
  d_b      = W_exp_b.T @ means_b - feats_b      (PE, accumulated in PSUM)
  a        = |d|          -> Sum_a  (ACT Abs + accum)
  m        = min(a, 1)    -> Sum_m  (DVE tensor_scalar + accum)
  0.5*m*m                 -> Sum_mm (DVE tensor_tensor_reduce)
SmoothL1 identity:  sl1(d) = 0.5*min(|d|,1)^2 + |d| - min(|d|,1)
  => total = Sum_a - Sum_m + Sum_mm ; loss = total / (N*D)
"""

import numpy as np
import ml_dtypes

NUM_CAMS = 8
N_CORES = 8
F_HALF = 1024
QCHUNK = 512


# ----------------------------------------------------------------------------
# Host-side preprocessing (index/int manipulation + row permutation only)
# ----------------------------------------------------------------------------

def _pack_core(rows, labs):
    """Greedy-pack whole label runs into blocks of <=128 rows whose label
    span is < 128. Returns list of (row_idx_array, local_slot_array, base)."""
    blocks = []
    n = len(rows)
    if n == 0:
        return blocks
    # run boundaries
    starts = np.flatnonzero(np.r_[True, labs[1:] != labs[:-1]])
    ends = np.r_[starts[1:], n]
    cur_s = None  # start row index of current block
    cur_used = 0
    cur_base = 0
    for s, e in zip(starts, ends):
        L = int(labs[s])
        rl = e - s
        if rl > 128:
            raise ValueError("label run longer than 128 rows not supported")
        if cur_s is None:
            cur_s, cur_used, cur_base = s, 0, L
        elif cur_used + rl > 128 or (L - cur_base) >= 128:
            blocks.append((rows[cur_s:cur_s + cur_used],
                           labs[cur_s:cur_s + cur_used] - cur_base, cur_base))
            cur_s, cur_used, cur_base = s, 0, L
        cur_used += rl
    if cur_s is not None and cur_used > 0:
        blocks.append((rows[cur_s:cur_s + cur_used],
                       labs[cur_s:cur_s + cur_used] - cur_base, cur_base))
    return blocks


def _preprocess(feats, labels, cam_ids):
    feats = np.ascontiguousarray(np.asarray(feats, dtype=np.float32))
    labels = np.asarray(labels).astype(np.int64)
    cams = np.asarray(cam_ids).astype(np.int64)
    N, D = feats.shape

    per_core_blocks = []
    for c in range(N_CORES):
        rows = np.flatnonzero(cams == c)
        order = np.argsort(labels[rows], kind="stable")
        rows = rows[order]
        labs = labels[rows]
        per_core_blocks.append(_pack_core(rows, labs))

    nblk = max(len(b) for b in per_core_blocks)
    nblk = max(nblk, 1)

    bf16 = ml_dtypes.bfloat16
    feats_s = np.zeros((N_CORES, nblk * 128, D), dtype=bf16)
    w_sum = np.zeros((N_CORES, nblk, 128, 128), dtype=bf16)
    w_exp = np.zeros((N_CORES, nblk, 128, 128), dtype=bf16)
    recip = np.ones((N_CORES, 128, nblk), dtype=np.float32)

    for c in range(N_CORES):
        for b, (ridx, slot, _base) in enumerate(per_core_blocks[c]):
            k = len(ridx)
            feats_s[c, 128 * b:128 * b + k] = feats[ridx]
            ar = np.arange(k)
            w_sum[c, b, ar, slot] = 1
            w_exp[c, b, slot, ar] = 1
            cnt = np.bincount(slot, minlength=128)
            recip[c, :, b] = np.float32(1.0) / np.maximum(cnt, 1).astype(np.float32)

    neg_ident = (-np.eye(128)).astype(bf16)
    return feats_s, w_sum, w_exp, recip, neg_ident, nblk, N, D


# ----------------------------------------------------------------------------
# Device program
# ----------------------------------------------------------------------------

def _build_program(nblk, D):
    import concourse.bacc as bacc
    import concourse.mybir as mybir
    import concourse.tile as tile

    dt = mybir.dt
    f32, bf16 = dt.float32, dt.bfloat16
    Alu = mybir.AluOpType
    Act = mybir.ActivationFunctionType
    n_half = D // F_HALF
    ncols = nblk * n_half

    nc = bacc.Bacc("TRN2", target_bir_lowering=False, debug=False,
                   num_devices=N_CORES)
    feats_d = nc.dram_tensor("feats_s", [nblk * 128, D], bf16,
                             kind="ExternalInput").ap()
    wsum_d = nc.dram_tensor("w_sum", [nblk, 128, 128], bf16,
                            kind="ExternalInput").ap()
    wexp_d = nc.dram_tensor("w_exp", [nblk, 128, 128], bf16,
                            kind="ExternalInput").ap()
    recip_d = nc.dram_tensor("recip", [128, nblk], f32,
                             kind="ExternalInput").ap()
    nident_d = nc.dram_tensor("neg_ident", [128, 128], bf16,
                              kind="ExternalInput").ap()
    out_d = nc.dram_tensor("partial", [1, 1], f32, kind="ExternalOutput").ap()

    with tile.TileContext(nc) as tc:
        with (
            tc.tile_pool(name="const", bufs=1) as const_pool,
            tc.tile_pool(name="feats", bufs=5) as feats_pool,
            tc.tile_pool(name="wts", bufs=4) as wts_pool,
            tc.tile_pool(name="means", bufs=3) as means_pool,
            tc.tile_pool(name="aa", bufs=3) as a_pool,
            tc.tile_pool(name="mm", bufs=3) as m_pool,
            tc.tile_pool(name="scr", bufs=2) as scr_pool,
            tc.tile_pool(name="psums", bufs=2, space="PSUM") as psum_s_pool,
            tc.tile_pool(name="psumd", bufs=2, space="PSUM") as psum_d_pool,
        ):
            negI = const_pool.tile([128, 128], bf16, tag="negI")
            nc.sync.dma_start(negI[:], nident_d[:])
            recip_sb = const_pool.tile([128, nblk], f32, tag="recip")
            nc.sync.dma_start(recip_sb[:], recip_d[:])
            stats_a = const_pool.tile([128, ncols], f32, tag="stats_a")
            stats_f = const_pool.tile([128, ncols], f32, tag="stats_f")
            ones = const_pool.tile([128, 1], f32, tag="ones")
            nc.gpsimd.memset(ones[:], 1.0)

            for b in range(nblk):
                fe = feats_pool.tile([128, D], bf16, tag="fe")
                nc.sync.dma_start(fe[:], feats_d[128 * b:128 * (b + 1), :])
                ws = wts_pool.tile([128, 128], bf16, tag="ws")
                nc.sync.dma_start(ws[:], wsum_d[b])
                we = wts_pool.tile([128, 128], bf16, tag="we")
                nc.sync.dma_start(we[:], wexp_d[b])

                for h in range(n_half):
                    col = b * n_half + h
                    sums = psum_s_pool.tile([128, F_HALF], f32, tag="sums")
                    for q in range(F_HALF // QCHUNK):
                        lo = h * F_HALF + q * QCHUNK
                        nc.tensor.matmul(
                            sums[:, q * QCHUNK:(q + 1) * QCHUNK],
                            ws[:], fe[:, lo:lo + QCHUNK],
                            start=True, stop=True)

                    means = means_pool.tile([128, F_HALF], bf16, tag="means")
                    rap = recip_sb[:, b:b + 1]
                    # DVE is the busier engine: give it only ~1/3 of the
                    # means copies, ACT the rest.
                    if col % 3 == 0:
                        nc.vector.tensor_scalar(means[:], sums[:], rap, None,
                                                op0=Alu.mult)
                    else:
                        nc.scalar.activation(means[:], sums[:], Act.Copy,
                                             bias=0.0, scale=rap)

                    dps = psum_d_pool.tile([128, F_HALF], f32, tag="d")
                    for q in range(F_HALF // QCHUNK):
                        lo = h * F_HALF + q * QCHUNK
                        sl = slice(q * QCHUNK, (q + 1) * QCHUNK)
                        nc.tensor.matmul(dps[:, sl], we[:],
                                         means[:, sl],
                                         start=True, stop=False)
                        nc.tensor.matmul(dps[:, sl], negI[:],
                                         fe[:, lo:lo + QCHUNK],
                                         start=False, stop=True)

                    a = a_pool.tile([128, F_HALF], bf16, tag="a")
                    nc.scalar.activation(a[:], dps[:], Act.Abs,
                                         accum_out=stats_a[:, col:col + 1])
                    # m = min(a,1): no accum -> DVE 4x mode
                    m = m_pool.tile([128, F_HALF], bf16, tag="m")
                    nc.vector.tensor_scalar(m[:], a[:], 1.0, None,
                                            op0=Alu.min)
                    # fused: (m - 2) * m = m^2 - 2m, accum sum
                    sc = scr_pool.tile([128, F_HALF], bf16, tag="sc")
                    nc.vector.scalar_tensor_tensor(
                        sc[:], m[:], -2.0, m[:],
                        op0=Alu.add, op1=Alu.mult,
                        accum_out=stats_f[:, col:col + 1])

            # final: comb = red_a + 0.5*red_f ; partial = sum_p comb[p]
            red_a = const_pool.tile([128, 1], f32, tag="red_a")
            nc.vector.tensor_reduce(red_a[:], stats_a[:],
                                    axis=mybir.AxisListType.X, op=Alu.add)
            red_f = const_pool.tile([128, 1], f32, tag="red_f")
            nc.vector.tensor_reduce(red_f[:], stats_f[:],
                                    axis=mybir.AxisListType.X, op=Alu.add)
            comb = const_pool.tile([128, 1], f32, tag="comb")
            nc.vector.scalar_tensor_tensor(comb[:], red_f[:], 0.5, red_a[:],
                                           op0=Alu.mult, op1=Alu.add)
            fin = psum_s_pool.tile([1, 1], f32, tag="sums")
            nc.tensor.matmul(fin[:], comb[:], ones[:], start=True, stop=True)
            outsb = const_pool.tile([1, 1], f32, tag="outsb")
            nc.scalar.copy(outsb[:], fin[:])
            nc.sync.dma_start(out_d[:], outsb[:])

    nc.compile()
    return nc


_PROGRAM_CACHE = {}


def _get_program(nblk, D):
    key = (nblk, D)
    if key not in _PROGRAM_CACHE:
        _PROGRAM_CACHE[key] = _build_program(nblk, D)
    return _PROGRAM_CACHE[key]


def make_in_maps(feats, labels, cam_ids):
    """Host shard + program build; returns (nc, in_maps, N, D)."""
    feats_s, w_sum, w_exp, recip, neg_ident, nblk, N, D = _preprocess(
        feats, labels, cam_ids)
    nc = _get_program(nblk, D)
    in_maps = [
        {
            "feats_s": feats_s[c],
            "w_sum": w_sum[c],
            "w_exp": w_exp[c],
            "recip": recip[c],
            "neg_ident": neg_ident,
        }
        for c in range(N_CORES)
    ]
    return nc, in_maps, N, D


def kernel(feats, labels, cam_ids):
    from concourse.bass_utils import run_bass_kernel_spmd

    nc, in_maps, N, D = make_in_maps(feats, labels, cam_ids)
    res = run_bass_kernel_spmd(nc, in_maps, core_ids=list(range(N_CORES)))
    total = np.sum(
        np.array([res.results[c]["partial"][0, 0] for c in range(N_CORES)],
                 dtype=np.float64))
    return np.float32(total / (float(N) * float(D)))


# revision 10
# speedup vs baseline: 1.2968x; 1.0609x over previous
"""CamCenterLoss (segment-mean SmoothL1) on 8 Trainium2 NeuronCores.

Sharding strategy: shard by camera id (there are exactly 8 cameras and 8
cores), so every (label, cam) segment is fully local to one core and no
collective is needed. On the host we sort each core's rows by label and pack
whole label-runs into 128-row blocks; each block's labels then span < 128
consecutive values, so segment sums/means/targets are computed with small
per-block one-hot matmuls on the TensorEngine.

Per block b (128 rows, feature dim split in halves of 1024):
  sums_b   = W_sum_b.T @ feats_b                (PE, bf16 operands, f32 PSUM)
  means_b  = sums_b * recip_b                   (ACT or DVE, per-partition scale)
  d_b      = W_exp_b.T @ means_b - feats_b      (PE, accumulated in PSUM)
  a        = |d|          -> Sum_a  (ACT Abs + accum)
  m        = min(a, 1)    -> Sum_m  (DVE tensor_scalar + accum)
  0.5*m*m                 -> Sum_mm (DVE tensor_tensor_reduce)
SmoothL1 identity:  sl1(d) = 0.5*min(|d|,1)^2 + |d| - min(|d|,1)
  => total = Sum_a - Sum_m + Sum_mm ; loss = total / (N*D)
"""

import numpy as np
import ml_dtypes

NUM_CAMS = 8
N_CORES = 8
F_HALF = 1024
QCHUNK = 512


# ----------------------------------------------------------------------------
# Host-side preprocessing (index/int manipulation + row permutation only)
# ----------------------------------------------------------------------------

def _pack_core(rows, labs):
    """Greedy-pack whole label runs into blocks of <=128 rows whose label
    span is < 128. Returns list of (row_idx_array, local_slot_array, base)."""
    blocks = []
    n = len(rows)
    if n == 0:
        return blocks
    # run boundaries
    starts = np.flatnonzero(np.r_[True, labs[1:] != labs[:-1]])
    ends = np.r_[starts[1:], n]
    cur_s = None  # start row index of current block
    cur_used = 0
    cur_base = 0
    for s, e in zip(starts, ends):
        L = int(labs[s])
        rl = e - s
        if rl > 128:
            raise ValueError("label run longer than 128 rows not supported")
        if cur_s is None:
            cur_s, cur_used, cur_base = s, 0, L
        elif cur_used + rl > 128 or (L - cur_base) >= 128:
            blocks.append((rows[cur_s:cur_s + cur_used],
                           labs[cur_s:cur_s + cur_used] - cur_base, cur_base))
            cur_s, cur_used, cur_base = s, 0, L
        cur_used += rl
    if cur_s is not None and cur_used > 0:
        blocks.append((rows[cur_s:cur_s + cur_used],
                       labs[cur_s:cur_s + cur_used] - cur_base, cur_base))
    return blocks


def _preprocess(feats, labels, cam_ids):
    feats = np.ascontiguousarray(np.asarray(feats, dtype=np.float32))
    labels = np.asarray(labels).astype(np.int64)
    cams = np.asarray(cam_ids).astype(np.int64)
    N, D = feats.shape

    per_core_blocks = []
    for c in range(N_CORES):
        rows = np.flatnonzero(cams == c)
        order = np.argsort(labels[rows], kind="stable")
        rows = rows[order]
        labs = labels[rows]
        per_core_blocks.append(_pack_core(rows, labs))

    nblk = max(len(b) for b in per_core_blocks)
    nblk = max(nblk, 1)

    bf16 = ml_dtypes.bfloat16
    feats_s = np.zeros((N_CORES, nblk * 128, D), dtype=bf16)
    w_sum = np.zeros((N_CORES, nblk, 128, 128), dtype=bf16)
    w_exp = np.zeros((N_CORES, nblk, 128, 128), dtype=bf16)
    recip = np.ones((N_CORES, 128, nblk), dtype=np.float32)

    for c in range(N_CORES):
        for b, (ridx, slot, _base) in enumerate(per_core_blocks[c]):
            k = len(ridx)
            feats_s[c, 128 * b:128 * b + k] = feats[ridx]
            ar = np.arange(k)
            w_sum[c, b, ar, slot] = 1
            w_exp[c, b, slot, ar] = 1
            cnt = np.bincount(slot, minlength=128)
            recip[c, :, b] = np.float32(1.0) / np.maximum(cnt, 1).astype(np.float32)

    neg_ident = (-np.eye(128)).astype(bf16)
    return feats_s, w_sum, w_exp, recip, neg_ident, nblk, N, D


# ----------------------------------------------------------------------------
# Device program
# ----------------------------------------------------------------------------

def _build_program(nblk, D):
    import concourse.bacc as bacc
    import concourse.mybir as mybir
    import concourse.tile as tile

    dt = mybir.dt
    f32, bf16 = dt.float32, dt.bfloat16
    Alu = mybir.AluOpType
    Act = mybir.ActivationFunctionType
    n_half = D // F_HALF
    ncols = nblk

    nc = bacc.Bacc("TRN2", target_bir_lowering=False, debug=False,
                   num_devices=N_CORES)
    feats_d = nc.dram_tensor("feats_s", [nblk * 128, D], bf16,
                             kind="ExternalInput").ap()
    wsum_d = nc.dram_tensor("w_sum", [nblk, 128, 128], bf16,
                            kind="ExternalInput").ap()
    wexp_d = nc.dram_tensor("w_exp", [nblk, 128, 128], bf16,
                            kind="ExternalInput").ap()
    recip_d = nc.dram_tensor("recip", [128, nblk], f32,
                             kind="ExternalInput").ap()
    nident_d = nc.dram_tensor("neg_ident", [128, 128], bf16,
                              kind="ExternalInput").ap()
    out_d = nc.dram_tensor("partial", [1, 1], f32, kind="ExternalOutput").ap()

    with tile.TileContext(nc) as tc:
        with (
            tc.tile_pool(name="const", bufs=1) as const_pool,
            tc.tile_pool(name="feats", bufs=5) as feats_pool,
            tc.tile_pool(name="wts", bufs=4) as wts_pool,
            tc.tile_pool(name="means", bufs=3) as means_pool,
            tc.tile_pool(name="aa", bufs=3) as a_pool,
            tc.tile_pool(name="mm", bufs=3) as m_pool,
            tc.tile_pool(name="scr", bufs=2) as scr_pool,
            tc.tile_pool(name="psums", bufs=2, space="PSUM") as psum_s_pool,
            tc.tile_pool(name="psumd", bufs=1, space="PSUM") as psum_d_pool,
        ):
            negI = const_pool.tile([128, 128], bf16, tag="negI")
            nc.sync.dma_start(negI[:], nident_d[:])
            recip_sb = const_pool.tile([128, nblk], f32, tag="recip")
            nc.sync.dma_start(recip_sb[:], recip_d[:])
            stats_a = const_pool.tile([128, ncols], f32, tag="stats_a")
            stats_f = const_pool.tile([128, ncols], f32, tag="stats_f")
            ones = const_pool.tile([128, 1], f32, tag="ones")
            nc.gpsimd.memset(ones[:], 1.0)

            for b in range(nblk):
                ws = wts_pool.tile([128, 128], bf16, tag="ws")
                nc.sync.dma_start(ws[:], wsum_d[b])
                we = wts_pool.tile([128, 128], bf16, tag="we")
                nc.sync.dma_start(we[:], wexp_d[b])
                fe = feats_pool.tile([128, D], bf16, tag="fe")
                for h in range(n_half):
                    nc.sync.dma_start(
                        fe[:, h * F_HALF:(h + 1) * F_HALF],
                        feats_d[128 * b:128 * (b + 1),
                                h * F_HALF:(h + 1) * F_HALF])

                # d for the whole block lives in one 4-bank PSUM tile
                dps = psum_d_pool.tile([128, D], f32, tag="d")
                rap = recip_sb[:, b:b + 1]
                for h in range(n_half):
                    col = b * n_half + h
                    sums = psum_s_pool.tile([128, F_HALF], f32, tag="sums")
                    for q in range(F_HALF // QCHUNK):
                        lo = h * F_HALF + q * QCHUNK
                        nc.tensor.matmul(
                            sums[:, q * QCHUNK:(q + 1) * QCHUNK],
                            ws[:], fe[:, lo:lo + QCHUNK],
                            start=True, stop=True)

                    means = means_pool.tile([128, F_HALF], bf16, tag="means")
                    # DVE is the busier engine: give it ~1/4 of the means
                    # copies, ACT the rest.
                    if col % 4 == 0:
                        nc.vector.tensor_scalar(means[:], sums[:], rap, None,
                                                op0=Alu.mult)
                    else:
                        nc.scalar.activation(means[:], sums[:], Act.Copy,
                                             bias=0.0, scale=rap)

                    for q in range(F_HALF // QCHUNK):
                        lo = h * F_HALF + q * QCHUNK
                        sl = slice(lo, lo + QCHUNK)
                        nc.tensor.matmul(dps[:, sl], we[:],
                                         means[:, q * QCHUNK:(q + 1) * QCHUNK],
                                         start=True, stop=False)
                        nc.tensor.matmul(dps[:, sl], negI[:],
                                         fe[:, sl],
                                         start=False, stop=True)

                # one full-block elementwise chain (FD = D)
                a = a_pool.tile([128, D], bf16, tag="a")
                nc.scalar.activation(a[:], dps[:], Act.Abs,
                                     accum_out=stats_a[:, b:b + 1])
                # m = min(a,1): no accum -> DVE 4x mode
                m = m_pool.tile([128, D], bf16, tag="m")
                nc.vector.tensor_scalar(m[:], a[:], 1.0, None, op0=Alu.min)
                # fused: (m - 2) * m = m^2 - 2m, accum sum
                sc = scr_pool.tile([128, D], bf16, tag="sc")
                nc.vector.scalar_tensor_tensor(
                    sc[:], m[:], -2.0, m[:],
                    op0=Alu.add, op1=Alu.mult,
                    accum_out=stats_f[:, b:b + 1])

            # final: comb = red_a + 0.5*red_f ; partial = sum_p comb[p]
            red_a = const_pool.tile([128, 1], f32, tag="red_a")
            nc.vector.tensor_reduce(red_a[:], stats_a[:],
                                    axis=mybir.AxisListType.X, op=Alu.add)
            red_f = const_pool.tile([128, 1], f32, tag="red_f")
            nc.vector.tensor_reduce(red_f[:], stats_f[:],
                                    axis=mybir.AxisListType.X, op=Alu.add)
            comb = const_pool.tile([128, 1], f32, tag="comb")
            nc.vector.scalar_tensor_tensor(comb[:], red_f[:], 0.5, red_a[:],
                                           op0=Alu.mult, op1=Alu.add)
            fin = psum_s_pool.tile([1, 1], f32, tag="sums")
            nc.tensor.matmul(fin[:], comb[:], ones[:], start=True, stop=True)
            outsb = const_pool.tile([1, 1], f32, tag="outsb")
            nc.scalar.copy(outsb[:], fin[:])
            nc.sync.dma_start(out_d[:], outsb[:])

    nc.compile()
    return nc


_PROGRAM_CACHE = {}


def _get_program(nblk, D):
    key = (nblk, D)
    if key not in _PROGRAM_CACHE:
        _PROGRAM_CACHE[key] = _build_program(nblk, D)
    return _PROGRAM_CACHE[key]


def make_in_maps(feats, labels, cam_ids):
    """Host shard + program build; returns (nc, in_maps, N, D)."""
    feats_s, w_sum, w_exp, recip, neg_ident, nblk, N, D = _preprocess(
        feats, labels, cam_ids)
    nc = _get_program(nblk, D)
    in_maps = [
        {
            "feats_s": feats_s[c],
            "w_sum": w_sum[c],
            "w_exp": w_exp[c],
            "recip": recip[c],
            "neg_ident": neg_ident,
        }
        for c in range(N_CORES)
    ]
    return nc, in_maps, N, D


def kernel(feats, labels, cam_ids):
    from concourse.bass_utils import run_bass_kernel_spmd

    nc, in_maps, N, D = make_in_maps(feats, labels, cam_ids)
    res = run_bass_kernel_spmd(nc, in_maps, core_ids=list(range(N_CORES)))
    total = np.sum(
        np.array([res.results[c]["partial"][0, 0] for c in range(N_CORES)],
                 dtype=np.float64))
    return np.float32(total / (float(N) * float(D)))


# revision 11
# speedup vs baseline: 1.3175x; 1.0159x over previous
"""CamCenterLoss (segment-mean SmoothL1) on 8 Trainium2 NeuronCores.

Sharding strategy: shard by camera id (there are exactly 8 cameras and 8
cores), so every (label, cam) segment is fully local to one core and no
collective is needed. On the host we sort each core's rows by label and pack
whole label-runs into 128-row blocks; each block's labels then span < 128
consecutive values, so segment sums/means/targets are computed with small
per-block one-hot matmuls on the TensorEngine.

Per block b (128 rows, feature dim split in halves of 1024):
  sums_b   = W_sum_b.T @ feats_b                (PE, bf16 operands, f32 PSUM)
  means_b  = sums_b * recip_b                   (ACT or DVE, per-partition scale)
  d_b      = W_exp_b.T @ means_b - feats_b      (PE, accumulated in PSUM)
  a        = |d|          -> Sum_a  (ACT Abs + accum)
  m        = min(a, 1)    -> Sum_m  (DVE tensor_scalar + accum)
  0.5*m*m                 -> Sum_mm (DVE tensor_tensor_reduce)
SmoothL1 identity:  sl1(d) = 0.5*min(|d|,1)^2 + |d| - min(|d|,1)
  => total = Sum_a - Sum_m + Sum_mm ; loss = total / (N*D)
"""

import numpy as np
import ml_dtypes

NUM_CAMS = 8
N_CORES = 8
F_HALF = 1024
QCHUNK = 512


# ----------------------------------------------------------------------------
# Host-side preprocessing (index/int manipulation + row permutation only)
# ----------------------------------------------------------------------------

def _pack_core(rows, labs):
    """Greedy-pack whole label runs into blocks of <=128 rows whose label
    span is < 128. Returns list of (row_idx_array, local_slot_array, base)."""
    blocks = []
    n = len(rows)
    if n == 0:
        return blocks
    # run boundaries
    starts = np.flatnonzero(np.r_[True, labs[1:] != labs[:-1]])
    ends = np.r_[starts[1:], n]
    cur_s = None  # start row index of current block
    cur_used = 0
    cur_base = 0
    for s, e in zip(starts, ends):
        L = int(labs[s])
        rl = e - s
        if rl > 128:
            raise ValueError("label run longer than 128 rows not supported")
        if cur_s is None:
            cur_s, cur_used, cur_base = s, 0, L
        elif cur_used + rl > 128 or (L - cur_base) >= 128:
            blocks.append((rows[cur_s:cur_s + cur_used],
                           labs[cur_s:cur_s + cur_used] - cur_base, cur_base))
            cur_s, cur_used, cur_base = s, 0, L
        cur_used += rl
    if cur_s is not None and cur_used > 0:
        blocks.append((rows[cur_s:cur_s + cur_used],
                       labs[cur_s:cur_s + cur_used] - cur_base, cur_base))
    return blocks


def _preprocess(feats, labels, cam_ids):
    feats = np.ascontiguousarray(np.asarray(feats, dtype=np.float32))
    labels = np.asarray(labels).astype(np.int64)
    cams = np.asarray(cam_ids).astype(np.int64)
    N, D = feats.shape

    per_core_blocks = []
    for c in range(N_CORES):
        rows = np.flatnonzero(cams == c)
        order = np.argsort(labels[rows], kind="stable")
        rows = rows[order]
        labs = labels[rows]
        per_core_blocks.append(_pack_core(rows, labs))

    nblk = max(len(b) for b in per_core_blocks)
    nblk = max(nblk, 1)

    bf16 = ml_dtypes.bfloat16
    feats_s = np.zeros((N_CORES, nblk * 128, D), dtype=bf16)
    # w_comb[b]: [:, :128] = row->slot one-hot (sum matmul weights, K=rows);
    #            [:, 128:] = slot->row one-hot (expand weights, K=slots)
    w_comb = np.zeros((N_CORES, nblk, 128, 256), dtype=bf16)
    recip = np.ones((N_CORES, 128, nblk), dtype=np.float32)

    for c in range(N_CORES):
        for b, (ridx, slot, _base) in enumerate(per_core_blocks[c]):
            k = len(ridx)
            feats_s[c, 128 * b:128 * b + k] = feats[ridx]
            ar = np.arange(k)
            w_comb[c, b, ar, slot] = 1
            w_comb[c, b, slot, 128 + ar] = 1
            cnt = np.bincount(slot, minlength=128)
            recip[c, :, b] = np.float32(1.0) / np.maximum(cnt, 1).astype(np.float32)

    neg_ident = (-np.eye(128)).astype(bf16)
    return feats_s, w_comb, recip, neg_ident, nblk, N, D


# ----------------------------------------------------------------------------
# Device program
# ----------------------------------------------------------------------------

def _build_program(nblk, D):
    import concourse.bacc as bacc
    import concourse.mybir as mybir
    import concourse.tile as tile

    dt = mybir.dt
    f32, bf16 = dt.float32, dt.bfloat16
    Alu = mybir.AluOpType
    Act = mybir.ActivationFunctionType
    n_half = D // F_HALF
    ncols = nblk

    nc = bacc.Bacc("TRN2", target_bir_lowering=False, debug=False,
                   num_devices=N_CORES)
    feats_d = nc.dram_tensor("feats_s", [nblk * 128, D], bf16,
                             kind="ExternalInput").ap()
    wcomb_d = nc.dram_tensor("w_comb", [nblk, 128, 256], bf16,
                             kind="ExternalInput").ap()
    recip_d = nc.dram_tensor("recip", [128, nblk], f32,
                             kind="ExternalInput").ap()
    nident_d = nc.dram_tensor("neg_ident", [128, 128], bf16,
                              kind="ExternalInput").ap()
    out_d = nc.dram_tensor("partial", [1, 1], f32, kind="ExternalOutput").ap()

    with tile.TileContext(nc) as tc:
        with (
            tc.tile_pool(name="const", bufs=1) as const_pool,
            tc.tile_pool(name="feats", bufs=5) as feats_pool,
            tc.tile_pool(name="wts", bufs=4) as wts_pool,
            tc.tile_pool(name="means", bufs=3) as means_pool,
            tc.tile_pool(name="aa", bufs=3) as a_pool,
            tc.tile_pool(name="mm", bufs=3) as m_pool,
            tc.tile_pool(name="scr", bufs=2) as scr_pool,
            tc.tile_pool(name="psums", bufs=2, space="PSUM") as psum_s_pool,
            tc.tile_pool(name="psumd", bufs=1, space="PSUM") as psum_d_pool,
        ):
            negI = const_pool.tile([128, 128], bf16, tag="negI")
            nc.sync.dma_start(negI[:], nident_d[:])
            recip_sb = const_pool.tile([128, nblk], f32, tag="recip")
            nc.sync.dma_start(recip_sb[:], recip_d[:])
            stats_a = const_pool.tile([128, ncols], f32, tag="stats_a")
            stats_f = const_pool.tile([128, ncols], f32, tag="stats_f")
            ones = const_pool.tile([128, 1], f32, tag="ones")
            nc.gpsimd.memset(ones[:], 1.0)

            for b in range(nblk):
                wc = wts_pool.tile([128, 256], bf16, tag="wc")
                nc.sync.dma_start(wc[:], wcomb_d[b])
                ws = wc[:, 0:128]
                we = wc[:, 128:256]
                fe = feats_pool.tile([128, D], bf16, tag="fe")
                nc.gpsimd.dma_start(fe[:], feats_d[128 * b:128 * (b + 1), :])

                # d for the whole block lives in one 4-bank PSUM tile
                dps = psum_d_pool.tile([128, D], f32, tag="d")
                rap = recip_sb[:, b:b + 1]
                for h in range(n_half):
                    col = b * n_half + h
                    sums = psum_s_pool.tile([128, F_HALF], f32, tag="sums")
                    for q in range(F_HALF // QCHUNK):
                        lo = h * F_HALF + q * QCHUNK
                        nc.tensor.matmul(
                            sums[:, q * QCHUNK:(q + 1) * QCHUNK],
                            ws, fe[:, lo:lo + QCHUNK],
                            start=True, stop=True)

                    means = means_pool.tile([128, F_HALF], bf16, tag="means")
                    # DVE is the busier engine: give it ~1/3 of the means
                    # copies, ACT the rest.
                    if col % 3 == 0:
                        nc.vector.tensor_scalar(means[:], sums[:], rap, None,
                                                op0=Alu.mult)
                    else:
                        nc.scalar.activation(means[:], sums[:], Act.Copy,
                                             bias=0.0, scale=rap)

                    for q in range(F_HALF // QCHUNK):
                        lo = h * F_HALF + q * QCHUNK
                        sl = slice(lo, lo + QCHUNK)
                        nc.tensor.matmul(dps[:, sl], negI[:],
                                         fe[:, sl],
                                         start=True, stop=False)
                        nc.tensor.matmul(dps[:, sl], we,
                                         means[:, q * QCHUNK:(q + 1) * QCHUNK],
                                         start=False, stop=True)

                # one full-block elementwise chain (FD = D)
                a = a_pool.tile([128, D], bf16, tag="a")
                nc.scalar.activation(a[:], dps[:], Act.Abs,
                                     accum_out=stats_a[:, b:b + 1])
                # m = min(a,1): no accum -> DVE 4x mode
                m = m_pool.tile([128, D], bf16, tag="m")
                nc.vector.tensor_scalar(m[:], a[:], 1.0, None, op0=Alu.min)
                # fused: (m - 2) * m = m^2 - 2m, accum sum
                sc = scr_pool.tile([128, D], bf16, tag="sc")
                nc.vector.scalar_tensor_tensor(
                    sc[:], m[:], -2.0, m[:],
                    op0=Alu.add, op1=Alu.mult,
                    accum_out=stats_f[:, b:b + 1])

            # final: comb = red_a + 0.5*red_f ; partial = sum_p comb[p]
            red_a = const_pool.tile([128, 1], f32, tag="red_a")
            nc.vector.tensor_reduce(red_a[:], stats_a[:],
                                    axis=mybir.AxisListType.X, op=Alu.add)
            red_f = const_pool.tile([128, 1], f32, tag="red_f")
            nc.vector.tensor_reduce(red_f[:], stats_f[:],
                                    axis=mybir.AxisListType.X, op=Alu.add)
            comb = const_pool.tile([128, 1], f32, tag="comb")
            nc.vector.scalar_tensor_tensor(comb[:], red_f[:], 0.5, red_a[:],
                                           op0=Alu.mult, op1=Alu.add)
            fin = psum_s_pool.tile([1, 1], f32, tag="sums")
            nc.tensor.matmul(fin[:], comb[:], ones[:], start=True, stop=True)
            outsb = const_pool.tile([1, 1], f32, tag="outsb")
            nc.scalar.copy(outsb[:], fin[:])
            nc.sync.dma_start(out_d[:], outsb[:])

    nc.compile()
    return nc


_PROGRAM_CACHE = {}


def _get_program(nblk, D):
    key = (nblk, D)
    if key not in _PROGRAM_CACHE:
        _PROGRAM_CACHE[key] = _build_program(nblk, D)
    return _PROGRAM_CACHE[key]


def make_in_maps(feats, labels, cam_ids):
    """Host shard + program build; returns (nc, in_maps, N, D)."""
    feats_s, w_comb, recip, neg_ident, nblk, N, D = _preprocess(
        feats, labels, cam_ids)
    nc = _get_program(nblk, D)
    in_maps = [
        {
            "feats_s": feats_s[c],
            "w_comb": w_comb[c],
            "recip": recip[c],
            "neg_ident": neg_ident,
        }
        for c in range(N_CORES)
    ]
    return nc, in_maps, N, D


def kernel(feats, labels, cam_ids):
    from concourse.bass_utils import run_bass_kernel_spmd

    nc, in_maps, N, D = make_in_maps(feats, labels, cam_ids)
    res = run_bass_kernel_spmd(nc, in_maps, core_ids=list(range(N_CORES)))
    total = np.sum(
        np.array([res.results[c]["partial"][0, 0] for c in range(N_CORES)],
                 dtype=np.float64))
    return np.float32(total / (float(N) * float(D)))
